# revision 28
# baseline (speedup 1.0000x reference)
"""Trainium2 Bass kernel for AttentionL2 (B=4, S=4096, DIN=384, DOUT=64).

out = softmax(cdist(q, k) / 8, axis=-1) @ v  with q/k/v = x @ W{q,k,v}.T

Sharding: 8 cores = 4 batches x 2 query-halves. Each core receives the
full x of its batch, host pre-transposed to x^T (bf16) with rows
reordered so its own query half comes first (softmax over keys is
permutation invariant). Every core runs the same SPMD program:
q = columns 0:2048, keys = all.

v2 restructuring vs the first working kernel (163.8us):
 - projections packed 2-wide: the stationary [Wq|Wk] / [Wk|Wv] pairs
   come for free as column slices of the same wT buffer, halving the
   projection matmul count for shared column ranges.
 - setup elementwise (copies, *-2, squares) moved to the ACT engine
   (copy/square live in every ACT table set, so they coexist with the
   Sqrt table at no switch cost); q2/k2 row sums via one 2-column
   ones-matmul over the [q;k] squared pair.
 - stage B (k-half projections) is interleaved chunk-by-chunk with
   phase-1 d2 tiles of the already-finished q-half region, keeping the
   PE warm (HAM) and removing the serial setup->phase1 boundary.
 - attention output accumulation (po) starts during phase 1 for tiles
   whose att came from the DVE composite path; a few DVE tiles are
   deferred into phase 2 so the DVE keeps working while ACT does Exp.

Per-core math (matmuls bf16 with fp32 accumulation):
  d2[j,i] = |q_i - k_j|^2 via one augmented matmul with contraction 66:
      lhsT = [-2*k^T; k2; 1; 0...]  (128 x 128 keys per tile)
      rhs  = [q^T; 1; q2; 0...]     (128 x 512)
  att = exp(sqrt(d2)/8) (unnormalized), two engine paths:
   - ScalarE: Sqrt(d2/64) -> fp16 buffer; after a scheduler barrier
     Exp with bias -2*ln(c0) -> bf16
   - VectorE: one custom DVE op (p(z)/c0)^2, p = minimax cubic of
     exp(sqrt(z)/16): the whole exp(sqrt(z)/8)/c0^2 in a single pass
  outT = [v; 1; 0...]^T @ att  (row 64 = softmax denominator, PSUM f32)
Final normalize outT[0:64]/outT[64] + transpose happen on the host.
"""

from contextlib import ExitStack

import ml_dtypes
import numpy as np

import concourse.bacc as bacc
import concourse.mybir as mybir
import concourse.tile as tile
from concourse import dve_ops
from concourse.dve_spec import Spec, Src0, C0, C1, C2, One, lower
from concourse.dve_uop import DveOpSpec
from concourse.bass_utils import run_bass_kernel_spmd

F32 = mybir.dt.float32
BF16 = mybir.dt.bfloat16
F16 = mybir.dt.float16
AF = mybir.ActivationFunctionType

B, S, DIN, DOUT = 4, 4096, 384, 64
M = S // 2        # query rows per core
KT = S // 128     # 32 key tiles
DC = DIN // 128   # 3 contraction chunks
NCORES = 8

# minimax cubic p for exp(sqrt(z)/16) on z in [32, 312], normalized by its
# constant term so the Horner tail can use the hardware One constant.
# att_dve = (p(z)/c0)^2 = exp(sqrt(z)/8)/c0^2; the ACT path matches the
# 1/c0^2 scale via a constant bias in its Exp (softmax is scale-invariant).
PA = 1.6518381642404523e-08
PB = -1.037933864407201e-05
PC = 0.006602996452846391
EXP_BIAS = -0.3424032850267295  # -2*ln(c0)

# tiles handled by the ACT sqrt/exp path (adjacent pairs so Exp can be
# batched as one [128,2,M] call); the rest use the DVE composite
ACT_TILES = (2, 3, 8, 9, 14, 15, 20, 21, 26, 27)



def _register_dve_op():
    name = "EXP_SQRT_SQ_ANT"
    if name in dve_ops._SUB_OPCODE_FOR_NAME:
        return next(op for op in dve_ops.OPS if op.name == name)
    t = ((Src0 * C0 + C1) * Src0 + C2) * Src0 + One
    body = t * t

    def ref(in0, in1, c0, c1, c2):
        tt = ((in0 * c0 + c1) * in0 + c2) * in0 + 1.0
        return tt * tt

    spec = Spec(body=body, reference=ref)
    row = max(dve_ops._SUB_OPCODE_FOR_NAME.values()) + 1
    assert row < 0x20
    dve_ops._SUB_OPCODE_FOR_NAME[name] = row
    shas = {}
    for ver in ("v3", "v4"):
        try:
            uops = lower(spec, ver=ver)
            shas[ver] = DveOpSpec(
                name=name, opcode=row, uops=uops, rd1_en=False
            ).sha(ver)
        except Exception:
            pass
    op = dve_ops.DveOp(name, spec, subdim=False, uops_sha=shas)
    dve_ops.OPS.append(op)
    dve_ops.CUSTOM_DVE_SPECS[name] = spec
    return op


EXP_OP = _register_dve_op()


def _is_act_tile(t):
    return t in ACT_TILES


def _body(tc, xt, wt, out):
    nc = tc.nc
    act_tiles = [t for t in range(KT) if _is_act_tile(t)]
    dve_tiles = [t for t in range(KT) if not _is_act_tile(t)]

    with ExitStack() as ctx:
        const_pool = ctx.enter_context(tc.tile_pool(name="const", bufs=1))
        # onesQ2: both columns select rows 0:64 (the q half of a squared
        # [q;k] pair) -> a [2,512] q2 result whose rows are identical, so
        # it can be copied to qT_aug[64:66] (base-partition-64 aligned);
        # row 64 is re-memset to 1 afterwards (the ones row).
        onesQ2 = const_pool.tile([128, 2], BF16)
        nc.vector.memset(onesQ2[:], 0.0)
        nc.vector.memset(onesQ2[0:64, :], 1.0)
        # onesK1: selects rows 64:128 (the k half) -> [1,512] k2 at
        # partition 0, copied to kT_aug[64:65].
        onesK1 = const_pool.tile([128, 1], BF16)
        nc.vector.memset(onesK1[:], 0.0)
        nc.vector.memset(onesK1[64:128, :], 1.0)
        ebias = const_pool.tile([128, 1], F32)
        nc.vector.memset(ebias[:], EXP_BIAS)

        main_pool = ctx.enter_context(tc.tile_pool(name="main", bufs=1))
        kT_aug = main_pool.tile([128, S], BF16)
        qT_aug = main_pool.tile([128, M], BF16)
        v_sb = main_pool.tile([128, KT, 128], BF16)
        # shared dist/att buffer: fp16 dist (ACT tiles) or bf16 att (DVE)
        buf = main_pool.tile([128, KT, M], F16)
        nc.vector.memset(kT_aug[64:128, :], 0.0)
        # rows 64:66 = 1.0; row 64 is overwritten by k2 per chunk, row 65
        # stays as the ones row (single-row memset at 65 is not a legal
        # base partition, hence the 2-row write)
        nc.vector.memset(kT_aug[64:66, :], 1.0)
        nc.vector.memset(qT_aug[64:128, :], 0.0)
        nc.gpsimd.memset(v_sb[:, :, 64:128], 0.0)
        nc.gpsimd.memset(v_sb[:, :, 64:65], 1.0)

        # PSUM pools: pp(3) + pb(1) + ps(4) = 8 banks during setup;
        # pp/pb close before po(4) opens, ps(4) stays -> 8 banks again.
        ps_pool = ctx.enter_context(tc.tile_pool(name="ps", bufs=2, space="PSUM"))

        def emit_d2_tile(t):
            for h in range(2):
                ps = ps_pool.tile([128, 1024], F32)
                base = h * 1024
                for s2 in range(2):
                    nc.tensor.matmul(
                        ps[:, s2 * 512 : (s2 + 1) * 512],
                        kT_aug[:, t * 128 : (t + 1) * 128],
                        qT_aug[:, base + s2 * 512 : base + (s2 + 1) * 512],
                        start=True,
                        stop=True,
                    )
                if _is_act_tile(t):
                    nc.scalar.activation(
                        buf[:, t, base : base + 1024], ps[:], AF.Sqrt,
                        scale=1.0 / 64.0,
                    )
                else:
                    nc.vector._custom_dve(
                        EXP_OP,
                        out=buf[:, t, base : base + 1024].bitcast(BF16),
                        in0=ps[:],
                        s0=PA,
                        s1=PB,
                        imm2=PC,
                    )

        po = None
        po_emitted = [0]
        PO_TOTAL = KT

        def emit_po_tile(t, att_ap):
            first = po_emitted[0] == 0
            last = po_emitted[0] == PO_TOTAL - 1
            for s2 in range(4):
                nc.tensor.matmul(
                    po[:, s2 * 512 : (s2 + 1) * 512],
                    v_sb[:, t, 0:128],
                    att_ap[:, s2 * 512 : (s2 + 1) * 512],
                    start=first,
                    stop=last,
                )
            po_emitted[0] += 1

        with ExitStack() as sctx:
            sb_pool = sctx.enter_context(tc.tile_pool(name="sbset", bufs=1))
            xk = sb_pool.tile([128, DC, M], BF16)
            wT = sb_pool.tile([128, DC, 3 * DOUT], BF16)
            # vT rows 0:64 = v of the q-half columns (from the v-only
            # matmul), rows 64:128 = v of the k-half columns (from the
            # [k;v] pair matmul) -- both partition-aligned copies.
            vT = sb_pool.tile([128, S], BF16)
            sq = sb_pool.tile([128, S], BF16, tag="sq")

            pp_pool = sctx.enter_context(
                tc.tile_pool(name="pp", bufs=2, space="PSUM")
            )
            pb_pool = sctx.enter_context(
                tc.tile_pool(name="pb", bufs=2, space="PSUM")
            )

            xt_r = xt.rearrange("(c p) s -> p c s", p=128)
            wt_r = wt.rearrange("(c p) w -> p c w", p=128)

            # ---------------- stage A: q-half projections ----------------
            with ExitStack() as actx:
                xq_pool = actx.enter_context(tc.tile_pool(name="xq", bufs=1))
                xq = xq_pool.tile([128, DC, M], BF16)

                # DMA policy: the ACT queue carries almost no dma (a HWDGE
                # issue occupies the queue for the transfer duration and
                # starves the setup copies) -- only the small wT load, done
                # by ~2.5us. The first-needed x pieces ride the sync ring;
                # the later halves go via GPSIMD/SWDGE in parallel. The 32
                # v transposes and the output also use sync.
                nc.scalar.dma_start(wT[:, :, :], wt_r[:, :, :])
                for c in range(DC):
                    nc.sync.dma_start(xq[:, c, 0:1024], xt_r[:, c, 0:1024])
                for c in range(DC):
                    nc.gpsimd.dma_start(
                        xq[:, c, 1024:2048], xt_r[:, c, 1024:2048]
                    )
                for c in range(DC):
                    nc.sync.dma_start(xk[:, c, 0:1024], xt_r[:, c, M : M + 1024])
                for c in range(DC):
                    nc.gpsimd.dma_start(
                        xk[:, c, 1024:2048], xt_r[:, c, M + 1024 : S]
                    )

                def emit_a_sum(ss):
                    # q2/k2 sums one chunk behind the projection chain: the
                    # squares are already done, so the PE never stalls here,
                    # and each 512-column slice of the aug rows unblocks its
                    # d2 tiles independently.
                    sl = slice(ss * 512, (ss + 1) * 512)
                    pbq = pb_pool.tile([2, 512], F32, tag="b")
                    nc.tensor.matmul(
                        pbq[:], onesQ2[:, 0:2], sq[:, sl], start=True, stop=True
                    )
                    # both rows = q2; row 64 re-memset to 1 per chunk
                    nc.scalar.copy(qT_aug[64:66, sl], pbq[:])
                    nc.vector.memset(qT_aug[64:65, sl], 1.0)
                    pbk = pb_pool.tile([1, 512], F32, tag="b")
                    nc.tensor.matmul(
                        pbk[:], onesK1[:, 0:1], sq[:, sl], start=True, stop=True
                    )
                    nc.scalar.copy(kT_aug[64:65, sl], pbk[0:1, :])

                for ss in range(4):
                    sl = slice(ss * 512, (ss + 1) * 512)
                    ppA = pp_pool.tile([128, 512], F32, tag="p")
                    for c in range(DC):
                        nc.tensor.matmul(
                            ppA[:], wT[:, c, 0:128], xq[:, c, sl],
                            start=(c == 0), stop=(c == DC - 1),
                        )
                    # rows 0:64 = q, 64:128 = k
                    nc.scalar.copy(qT_aug[0:64, sl], ppA[0:64, :])
                    nc.scalar.mul(kT_aug[0:64, sl], ppA[64:128, :], -2.0)
                    nc.scalar.square(sq[:, sl], ppA[:])

                    ppC = pp_pool.tile([64, 512], F32, tag="p")
                    for c in range(DC):
                        nc.tensor.matmul(
                            ppC[:], wT[:, c, 128:192], xq[:, c, sl],
                            start=(c == 0), stop=(c == DC - 1),
                        )
                    nc.scalar.copy(vT[0:64, sl], ppC[:])
                    for j in range(4):
                        t = ss * 4 + j
                        nc.sync.dma_start_transpose(
                            v_sb[:, t, 0:64], vT[0:64, t * 128 : (t + 1) * 128]
                        )
                    if ss > 0:
                        emit_a_sum(ss - 1)
                emit_a_sum(3)

            # ------- stage B: k-half projections ||| phase-1 tiles 0..15 -------
            early_po_q = []  # DVE tiles whose att is ready for early po

            def emit_b_sum(ss):
                sl = slice(M + ss * 512, M + (ss + 1) * 512)
                pb = pb_pool.tile([1, 512], F32, tag="b")
                nc.tensor.matmul(
                    pb[:], onesQ2[0:64, 0:1], sq[0:64, sl], start=True, stop=True
                )
                nc.scalar.copy(kT_aug[64:65, sl], pb[0:1, :])

            for ss in range(4):
                sl = slice(M + ss * 512, M + (ss + 1) * 512)
                dsl = slice(ss * 512, (ss + 1) * 512)
                ppA = pp_pool.tile([128, 512], F32, tag="p")
                for c in range(DC):
                    nc.tensor.matmul(
                        ppA[:], wT[:, c, 64:192], xk[:, c, dsl],
                        start=(c == 0), stop=(c == DC - 1),
                    )
                # rows 0:64 = k, 64:128 = v
                nc.scalar.mul(kT_aug[0:64, sl], ppA[0:64, :], -2.0)
                nc.scalar.copy(vT[64:128, sl], ppA[64:128, :])
                nc.scalar.square(sq[0:64, sl], ppA[0:64, :])
                # k2 sum for the PREVIOUS chunk (its square is long done,
                # so the PE does not stall on the ACT chain)
                if ss > 0:
                    emit_b_sum(ss - 1)
                for j in range(4):
                    t = 16 + ss * 4 + j
                    nc.sync.dma_start_transpose(
                        v_sb[:, t, 0:64], vT[64:128, t * 128 : (t + 1) * 128]
                    )
                # phase-1 tiles over the q-half key region; ACT tiles first
                # so all sqrts finish early and the Exp phase can overlap
                # the remaining DVE-composite stretch
                quad = [ss * 4 + j for j in range(4)]
                for t in sorted(quad, key=lambda u: not _is_act_tile(u)):
                    emit_d2_tile(t)
                    if not _is_act_tile(t):
                        early_po_q.append(t)
            emit_b_sum(3)

        # setup pools closed (pp/pb PSUM freed) -> open po
        po_pool = ctx.enter_context(tc.tile_pool(name="po", bufs=1, space="PSUM"))
        po = po_pool.tile([128, M], F32)

        pending_po = list(early_po_q)

        def pop_po(n):
            for _ in range(n):
                if pending_po:
                    c = pending_po.pop(0)
                    emit_po_tile(c, buf[:, c, :].bitcast(BF16))

        # ------- rest of the tile stream: remaining ACT tiles first
        # (interleaved with DVE tiles for ps rotation), then the DVE tail;
        # po groups for ready DVE tiles fill PE slack throughout -------
        rest_act = [t for t in act_tiles if t >= 16]
        rest_dve = [t for t in dve_tiles if t >= 16]
        order = []
        for i, a in enumerate(rest_act):
            order.append(a)
            if i < len(rest_dve):
                order.append(rest_dve[i])
        order += rest_dve[len(rest_act):]
        for t in order:
            emit_d2_tile(t)
            if not _is_act_tile(t):
                pending_po.append(t)
            pop_po(1)

        tc.no_sync_barrier()  # all Sqrt before all Exp: one table switch

        # ------- Exp pairs overlap the remaining DVE composites -------
        with ExitStack() as p2ctx:
            att_pool = p2ctx.enter_context(tc.tile_pool(name="att", bufs=3))
            pairs = [
                (act_tiles[i], act_tiles[i + 1])
                for i in range(0, len(act_tiles), 2)
            ]
            for t0, t1 in pairs:
                att2 = att_pool.tile([128, 2, M], BF16)
                nc.scalar.activation(
                    att2[:], buf[:, t0 : t0 + 2, :], AF.Exp, bias=ebias[:]
                )
                emit_po_tile(t0, att2[:, 0, :])
                emit_po_tile(t1, att2[:, 1, :])
                pop_po(2)
            pop_po(len(pending_po))
            assert po_emitted[0] == PO_TOTAL

            # -------- finish: copy outT[0:65] to SBUF, DMA out --------
            # split across DVE and ACT to halve the tail
            oT_pool = p2ctx.enter_context(tc.tile_pool(name="oT", bufs=1))
            oT = oT_pool.tile([65, M], F32)
            nc.vector.tensor_copy(oT[:, 0 : M // 2], po[0:65, 0 : M // 2])
            nc.scalar.copy(oT[:, M // 2 : M], po[0:65, M // 2 : M])
            nc.sync.dma_start(out[:, 0 : M // 2], oT[:, 0 : M // 2])
            nc.sync.dma_start(out[:, M // 2 : M], oT[:, M // 2 : M])


_NC_CACHE = None


def build():
    global _NC_CACHE
    if _NC_CACHE is not None:
        return _NC_CACHE
    nc = bacc.Bacc("TRN2", target_bir_lowering=False, debug=False, num_devices=NCORES)
    xt_d = nc.declare_dram_parameter("xt", [DIN, S], BF16, isOutput=False)
    wt_d = nc.declare_dram_parameter("wt", [DIN, 3 * DOUT], BF16, isOutput=False)
    out_d = nc.declare_dram_parameter("out", [65, M], F32, isOutput=True)
    with tile.TileContext(nc) as tc:
        _body(tc, xt_d[:], wt_d[:], out_d[:])
    nc.compile()
    _NC_CACHE = nc
    return nc


def make_in_maps(x, Wq, Wk, Wv):
    bf16 = ml_dtypes.bfloat16
    wt = np.ascontiguousarray(
        np.concatenate(
            [np.asarray(W, np.float32).T for W in (Wq, Wk, Wv)], axis=1
        ).astype(bf16)
    )
    in_maps = []
    for c in range(NCORES):
        b, h = divmod(c, 2)
        xb = np.asarray(x[b], np.float32)
        xc = np.concatenate(
            [xb[h * M : (h + 1) * M], xb[(1 - h) * M : (2 - h) * M]], 0
        )
        in_maps.append({"xt": np.ascontiguousarray(xc.T.astype(bf16)), "wt": wt})
    return in_maps


def gather_out(results):
    out = np.zeros((B, S, DOUT), np.float32)
    for c in range(NCORES):
        b, h = divmod(c, 2)
        oT = np.asarray(results[c]["out"], np.float32)
        out[b, h * M : (h + 1) * M] = (oT[0:64] / oT[64:65]).T
    return out


def kernel(x, Wq, Wk, Wv):
    nc = build()
    in_maps = make_in_maps(x, Wq, Wk, Wv)
    res = run_bass_kernel_spmd(nc, in_maps, core_ids=list(range(NCORES)))
    return gather_out(res.results)


# revision 29
# speedup vs baseline: 1.1799x; 1.1799x over previous
"""Trainium2 Bass kernel for AttentionL2 (B=4, S=4096, DIN=384, DOUT=64).

out = softmax(cdist(q, k) / 8, axis=-1) @ v  with q/k/v = x @ W{q,k,v}.T

Sharding: 8 cores = 4 batches x 2 query-halves. Each core receives the
full x of its batch, host pre-transposed to x^T (bf16) with rows
reordered so its own query half comes first (softmax over keys is
permutation invariant). Every core runs the same SPMD program:
q = columns 0:2048, keys = all.

v2 restructuring vs the first working kernel (163.8us):
 - projections packed 2-wide: the stationary [Wq|Wk] / [Wk|Wv] pairs
   come for free as column slices of the same wT buffer, halving the
   projection matmul count for shared column ranges.
 - setup elementwise (copies, *-2, squares) moved to the ACT engine
   (copy/square live in every ACT table set, so they coexist with the
   Sqrt table at no switch cost); q2/k2 row sums via one 2-column
   ones-matmul over the [q;k] squared pair.
 - stage B (k-half projections) is interleaved chunk-by-chunk with
   phase-1 d2 tiles of the already-finished q-half region, keeping the
   PE warm (HAM) and removing the serial setup->phase1 boundary.
 - attention output accumulation (po) starts during phase 1 for tiles
   whose att came from the DVE composite path; a few DVE tiles are
   deferred into phase 2 so the DVE keeps working while ACT does Exp.

Per-core math (matmuls bf16 with fp32 accumulation):
  d2[j,i] = |q_i - k_j|^2 via one augmented matmul with contraction 66:
      lhsT = [-2*k^T; k2; 1; 0...]  (128 x 128 keys per tile)
      rhs  = [q^T; 1; q2; 0...]     (128 x 512)
  att = exp(sqrt(d2)/8) (unnormalized), two engine paths:
   - ScalarE: Sqrt(d2/64) -> fp16 buffer; after a scheduler barrier
     Exp with bias -2*ln(c0) -> bf16
   - VectorE: one custom DVE op (p(z)/c0)^2, p = minimax cubic of
     exp(sqrt(z)/16): the whole exp(sqrt(z)/8)/c0^2 in a single pass
  outT = [v; 1; 0...]^T @ att  (row 64 = softmax denominator, PSUM f32)
Final normalize outT[0:64]/outT[64] + transpose happen on the host.
"""

from contextlib import ExitStack

import ml_dtypes
import numpy as np

import concourse.bacc as bacc
import concourse.mybir as mybir
import concourse.tile as tile
from concourse import dve_ops
from concourse.dve_spec import Spec, Src0, C0, C1, C2, One, lower
from concourse.dve_uop import DveOpSpec
from concourse.bass_utils import run_bass_kernel_spmd

F32 = mybir.dt.float32
BF16 = mybir.dt.bfloat16
F16 = mybir.dt.float16
AF = mybir.ActivationFunctionType

B, S, DIN, DOUT = 4, 4096, 384, 64
M = S // 2        # query rows per core
KT = S // 128     # 32 key tiles
DC = DIN // 128   # 3 contraction chunks
NCORES = 8

# minimax cubic p for exp(sqrt(z)/16) on z in [32, 312], normalized by its
# constant term so the Horner tail can use the hardware One constant.
# att_dve = (p(z)/c0)^2 = exp(sqrt(z)/8)/c0^2; the ACT path matches the
# 1/c0^2 scale via a constant bias in its Exp (softmax is scale-invariant).
PA = 1.6518381642404523e-08
PB = -1.037933864407201e-05
PC = 0.006602996452846391
EXP_BIAS = -0.3424032850267295  # -2*ln(c0)

# tiles handled by the ACT sqrt/exp path (adjacent pairs so Exp can be
# batched as one [128,2,M] call); the rest use the DVE composite
ACT_TILES = (2, 3, 8, 9, 14, 15, 20, 21, 26, 27)



def _register_dve_op():
    name = "EXP_SQRT_SQ_ANT"
    if name in dve_ops._SUB_OPCODE_FOR_NAME:
        return next(op for op in dve_ops.OPS if op.name == name)
    t = ((Src0 * C0 + C1) * Src0 + C2) * Src0 + One
    body = t * t

    def ref(in0, in1, c0, c1, c2):
        tt = ((in0 * c0 + c1) * in0 + c2) * in0 + 1.0
        return tt * tt

    spec = Spec(body=body, reference=ref)
    row = max(dve_ops._SUB_OPCODE_FOR_NAME.values()) + 1
    assert row < 0x20
    dve_ops._SUB_OPCODE_FOR_NAME[name] = row
    shas = {}
    for ver in ("v3", "v4"):
        try:
            uops = lower(spec, ver=ver)
            shas[ver] = DveOpSpec(
                name=name, opcode=row, uops=uops, rd1_en=False
            ).sha(ver)
        except Exception:
            pass
    op = dve_ops.DveOp(name, spec, subdim=False, uops_sha=shas)
    dve_ops.OPS.append(op)
    dve_ops.CUSTOM_DVE_SPECS[name] = spec
    return op


EXP_OP = _register_dve_op()


def _is_act_tile(t):
    return t in ACT_TILES


def _body(tc, xt, wt, out):
    nc = tc.nc
    act_tiles = [t for t in range(KT) if _is_act_tile(t)]
    dve_tiles = [t for t in range(KT) if not _is_act_tile(t)]

    with ExitStack() as ctx:
        const_pool = ctx.enter_context(tc.tile_pool(name="const", bufs=1))
        # onesQ2: both columns select rows 0:64 (the q half of a squared
        # [q;k] pair) -> a [2,512] q2 result whose rows are identical, so
        # it can be copied to qT_aug[64:66] (base-partition-64 aligned);
        # row 64 is re-memset to 1 afterwards (the ones row).
        onesQ2 = const_pool.tile([128, 2], BF16)
        nc.vector.memset(onesQ2[:], 0.0)
        nc.vector.memset(onesQ2[0:64, :], 1.0)
        # onesK1: selects rows 64:128 (the k half) -> [1,512] k2 at
        # partition 0, copied to kT_aug[64:65].
        onesK1 = const_pool.tile([128, 1], BF16)
        nc.vector.memset(onesK1[:], 0.0)
        nc.vector.memset(onesK1[64:128, :], 1.0)
        ebias = const_pool.tile([128, 1], F32)
        nc.vector.memset(ebias[:], EXP_BIAS)

        main_pool = ctx.enter_context(tc.tile_pool(name="main", bufs=1))
        kT_aug = main_pool.tile([128, S], BF16)
        qT_aug = main_pool.tile([128, M], BF16)
        v_sb = main_pool.tile([128, KT, 128], BF16)
        # shared dist/att buffer: fp16 dist (ACT tiles) or bf16 att (DVE)
        buf = main_pool.tile([128, KT, M], F16)
        nc.vector.memset(kT_aug[64:128, :], 0.0)
        # rows 64:66 = 1.0; row 64 is overwritten by k2 per chunk, row 65
        # stays as the ones row (single-row memset at 65 is not a legal
        # base partition, hence the 2-row write)
        nc.vector.memset(kT_aug[64:66, :], 1.0)
        nc.vector.memset(qT_aug[64:128, :], 0.0)
        nc.gpsimd.memset(v_sb[:, :, 64:128], 0.0)
        nc.gpsimd.memset(v_sb[:, :, 64:65], 1.0)

        # PSUM pools: pp(3) + pb(1) + ps(4) = 8 banks during setup;
        # pp/pb close before po(4) opens, ps(4) stays -> 8 banks again.
        ps_pool = ctx.enter_context(tc.tile_pool(name="ps", bufs=2, space="PSUM"))

        def emit_d2_tile(t):
            for h in range(2):
                ps = ps_pool.tile([128, 1024], F32)
                base = h * 1024
                for s2 in range(2):
                    nc.tensor.matmul(
                        ps[:, s2 * 512 : (s2 + 1) * 512],
                        kT_aug[:, t * 128 : (t + 1) * 128],
                        qT_aug[:, base + s2 * 512 : base + (s2 + 1) * 512],
                        start=True,
                        stop=True,
                    )
                if _is_act_tile(t):
                    nc.scalar.activation(
                        buf[:, t, base : base + 1024], ps[:], AF.Sqrt,
                        scale=1.0 / 64.0,
                    )
                else:
                    nc.vector._custom_dve(
                        EXP_OP,
                        out=buf[:, t, base : base + 1024].bitcast(BF16),
                        in0=ps[:],
                        s0=PA,
                        s1=PB,
                        imm2=PC,
                    )

        po = None
        po_emitted = [0]
        PO_TOTAL = KT

        def emit_po_tile(t, att_ap):
            first = po_emitted[0] == 0
            last = po_emitted[0] == PO_TOTAL - 1
            for s2 in range(4):
                nc.tensor.matmul(
                    po[:, s2 * 512 : (s2 + 1) * 512],
                    v_sb[:, t, 0:128],
                    att_ap[:, s2 * 512 : (s2 + 1) * 512],
                    start=first,
                    stop=last,
                )
            po_emitted[0] += 1

        with ExitStack() as sctx:
            sb_pool = sctx.enter_context(tc.tile_pool(name="sbset", bufs=1))
            xk = sb_pool.tile([128, DC, M], BF16)
            wT = sb_pool.tile([128, DC, 3 * DOUT], BF16)
            # vT rows 0:64 = v of the q-half columns (from the v-only
            # matmul), rows 64:128 = v of the k-half columns (from the
            # [k;v] pair matmul) -- both partition-aligned copies.
            vT = sb_pool.tile([128, S], BF16)
            sq = sb_pool.tile([128, S], BF16, tag="sq")

            pp_pool = sctx.enter_context(
                tc.tile_pool(name="pp", bufs=2, space="PSUM")
            )
            pb_pool = sctx.enter_context(
                tc.tile_pool(name="pb", bufs=2, space="PSUM")
            )

            xt_r = xt.rearrange("(c p) s -> p c s", p=128)
            wt_r = wt.rearrange("(c p) w -> p c w", p=128)

            # ---------------- stage A: q-half projections ----------------
            with ExitStack() as actx:
                xq_pool = actx.enter_context(tc.tile_pool(name="xq", bufs=1))
                xq = xq_pool.tile([128, DC, M], BF16)

                # DMA policy: the ACT queue carries almost no dma (a HWDGE
                # issue occupies the queue for the transfer duration and
                # starves the setup copies) -- only the small wT load, done
                # by ~2.5us. The first-needed x pieces ride the sync ring;
                # the later halves go via GPSIMD/SWDGE in parallel. The 32
                # v transposes and the output also use sync.
                nc.scalar.dma_start(wT[:, :, :], wt_r[:, :, :])
                for c in range(DC):
                    nc.sync.dma_start(xq[:, c, 0:1024], xt_r[:, c, 0:1024])
                for c in range(DC):
                    nc.gpsimd.dma_start(
                        xq[:, c, 1024:2048], xt_r[:, c, 1024:2048]
                    )
                for c in range(DC):
                    nc.gpsimd.dma_start(xk[:, c, 0:1024], xt_r[:, c, M : M + 1024])
                for c in range(DC):
                    nc.gpsimd.dma_start(
                        xk[:, c, 1024:2048], xt_r[:, c, M + 1024 : S]
                    )

                def emit_a_sum(ss):
                    # q2/k2 sums one chunk behind the projection chain: the
                    # squares are already done, so the PE never stalls here,
                    # and each 512-column slice of the aug rows unblocks its
                    # d2 tiles independently.
                    sl = slice(ss * 512, (ss + 1) * 512)
                    pbq = pb_pool.tile([2, 512], F32, tag="b")
                    nc.tensor.matmul(
                        pbq[:], onesQ2[:, 0:2], sq[:, sl], start=True, stop=True
                    )
                    # both rows = q2; row 64 re-memset to 1 per chunk
                    nc.scalar.copy(qT_aug[64:66, sl], pbq[:])
                    nc.vector.memset(qT_aug[64:65, sl], 1.0)
                    pbk = pb_pool.tile([1, 512], F32, tag="b")
                    nc.tensor.matmul(
                        pbk[:], onesK1[:, 0:1], sq[:, sl], start=True, stop=True
                    )
                    nc.scalar.copy(kT_aug[64:65, sl], pbk[0:1, :])

                for ss in range(4):
                    sl = slice(ss * 512, (ss + 1) * 512)
                    ppA = pp_pool.tile([128, 512], F32, tag="p")
                    for c in range(DC):
                        nc.tensor.matmul(
                            ppA[:], wT[:, c, 0:128], xq[:, c, sl],
                            start=(c == 0), stop=(c == DC - 1),
                        )
                    # rows 0:64 = q, 64:128 = k
                    nc.scalar.copy(qT_aug[0:64, sl], ppA[0:64, :])
                    nc.scalar.mul(kT_aug[0:64, sl], ppA[64:128, :], -2.0)
                    nc.scalar.square(sq[:, sl], ppA[:])

                    ppC = pp_pool.tile([64, 512], F32, tag="p")
                    for c in range(DC):
                        nc.tensor.matmul(
                            ppC[:], wT[:, c, 128:192], xq[:, c, sl],
                            start=(c == 0), stop=(c == DC - 1),
                        )
                    nc.scalar.copy(vT[0:64, sl], ppC[:])
                    for j in range(4):
                        t = ss * 4 + j
                        nc.sync.dma_start_transpose(
                            v_sb[:, t, 0:64], vT[0:64, t * 128 : (t + 1) * 128]
                        )
                    if ss > 0:
                        emit_a_sum(ss - 1)
                emit_a_sum(3)

            # ------- stage B: k-half projections ||| phase-1 tiles 0..15 -------
            early_po_q = []  # DVE tiles whose att is ready for early po

            def emit_b_sum(ss):
                sl = slice(M + ss * 512, M + (ss + 1) * 512)
                pb = pb_pool.tile([1, 512], F32, tag="b")
                nc.tensor.matmul(
                    pb[:], onesQ2[0:64, 0:1], sq[0:64, sl], start=True, stop=True
                )
                nc.scalar.copy(kT_aug[64:65, sl], pb[0:1, :])

            for ss in range(4):
                sl = slice(M + ss * 512, M + (ss + 1) * 512)
                dsl = slice(ss * 512, (ss + 1) * 512)
                ppA = pp_pool.tile([128, 512], F32, tag="p")
                for c in range(DC):
                    nc.tensor.matmul(
                        ppA[:], wT[:, c, 64:192], xk[:, c, dsl],
                        start=(c == 0), stop=(c == DC - 1),
                    )
                # rows 0:64 = k, 64:128 = v
                nc.scalar.mul(kT_aug[0:64, sl], ppA[0:64, :], -2.0)
                nc.scalar.copy(vT[64:128, sl], ppA[64:128, :])
                nc.scalar.square(sq[0:64, sl], ppA[0:64, :])
                # k2 sum for the PREVIOUS chunk (its square is long done,
                # so the PE does not stall on the ACT chain)
                if ss > 0:
                    emit_b_sum(ss - 1)
                for j in range(4):
                    t = 16 + ss * 4 + j
                    nc.sync.dma_start_transpose(
                        v_sb[:, t, 0:64], vT[64:128, t * 128 : (t + 1) * 128]
                    )
                # phase-1 tiles over the q-half key region; ACT tiles first
                # so all sqrts finish early and the Exp phase can overlap
                # the remaining DVE-composite stretch
                quad = [ss * 4 + j for j in range(4)]
                for t in sorted(quad, key=lambda u: not _is_act_tile(u)):
                    emit_d2_tile(t)
                    if not _is_act_tile(t):
                        early_po_q.append(t)
            emit_b_sum(3)

        # setup pools closed (pp/pb PSUM freed) -> open po
        po_pool = ctx.enter_context(tc.tile_pool(name="po", bufs=1, space="PSUM"))
        po = po_pool.tile([128, M], F32)

        pending_po = list(early_po_q)

        def pop_po(n):
            for _ in range(n):
                if pending_po:
                    c = pending_po.pop(0)
                    emit_po_tile(c, buf[:, c, :].bitcast(BF16))

        # ------- rest of the tile stream: remaining ACT tiles first
        # (interleaved with DVE tiles for ps rotation), then the DVE tail;
        # po groups for ready DVE tiles fill PE slack throughout -------
        rest_act = [t for t in act_tiles if t >= 16]
        rest_dve = [t for t in dve_tiles if t >= 16]
        order = []
        for i, a in enumerate(rest_act):
            order.append(a)
            if i < len(rest_dve):
                order.append(rest_dve[i])
        order += rest_dve[len(rest_act):]
        for t in order:
            emit_d2_tile(t)
            if not _is_act_tile(t):
                pending_po.append(t)
            pop_po(1)

        tc.no_sync_barrier()  # all Sqrt before all Exp: one table switch

        # ------- Exp pairs overlap the remaining DVE composites -------
        with ExitStack() as p2ctx:
            att_pool = p2ctx.enter_context(tc.tile_pool(name="att", bufs=3))
            pairs = [
                (act_tiles[i], act_tiles[i + 1])
                for i in range(0, len(act_tiles), 2)
            ]
            for t0, t1 in pairs:
                att2 = att_pool.tile([128, 2, M], BF16)
                nc.scalar.activation(
                    att2[:], buf[:, t0 : t0 + 2, :], AF.Exp, bias=ebias[:]
                )
                emit_po_tile(t0, att2[:, 0, :])
                emit_po_tile(t1, att2[:, 1, :])
                pop_po(2)
            pop_po(len(pending_po))
            assert po_emitted[0] == PO_TOTAL

            # -------- finish: copy outT[0:65] to SBUF, DMA out --------
            # split across DVE and ACT to halve the tail
            oT_pool = p2ctx.enter_context(tc.tile_pool(name="oT", bufs=1))
            oT = oT_pool.tile([65, M], F32)
            nc.vector.tensor_copy(oT[:, 0 : M // 2], po[0:65, 0 : M // 2])
            nc.scalar.copy(oT[:, M // 2 : M], po[0:65, M // 2 : M])
            nc.sync.dma_start(out[:, 0 : M // 2], oT[:, 0 : M // 2])
            nc.sync.dma_start(out[:, M // 2 : M], oT[:, M // 2 : M])


_NC_CACHE = None


def build():
    global _NC_CACHE
    if _NC_CACHE is not None:
        return _NC_CACHE
    nc = bacc.Bacc("TRN2", target_bir_lowering=False, debug=False, num_devices=NCORES)
    xt_d = nc.declare_dram_parameter("xt", [DIN, S], BF16, isOutput=False)
    wt_d = nc.declare_dram_parameter("wt", [DIN, 3 * DOUT], BF16, isOutput=False)
    out_d = nc.declare_dram_parameter("out", [65, M], F32, isOutput=True)
    with tile.TileContext(nc) as tc:
        _body(tc, xt_d[:], wt_d[:], out_d[:])
    nc.compile()
    _NC_CACHE = nc
    return nc


def make_in_maps(x, Wq, Wk, Wv):
    bf16 = ml_dtypes.bfloat16
    wt = np.ascontiguousarray(
        np.concatenate(
            [np.asarray(W, np.float32).T for W in (Wq, Wk, Wv)], axis=1
        ).astype(bf16)
    )
    in_maps = []
    for c in range(NCORES):
        b, h = divmod(c, 2)
        xb = np.asarray(x[b], np.float32)
        xc = np.concatenate(
            [xb[h * M : (h + 1) * M], xb[(1 - h) * M : (2 - h) * M]], 0
        )
        in_maps.append({"xt": np.ascontiguousarray(xc.T.astype(bf16)), "wt": wt})
    return in_maps


def gather_out(results):
    out = np.zeros((B, S, DOUT), np.float32)
    for c in range(NCORES):
        b, h = divmod(c, 2)
        oT = np.asarray(results[c]["out"], np.float32)
        out[b, h * M : (h + 1) * M] = (oT[0:64] / oT[64:65]).T
    return out


def kernel(x, Wq, Wk, Wv):
    nc = build()
    in_maps = make_in_maps(x, Wq, Wk, Wv)
    res = run_bass_kernel_spmd(nc, in_maps, core_ids=list(range(NCORES)))
    return gather_out(res.results)
